# revision 1
# baseline (speedup 1.0000x reference)
"""Trainium2 Bass kernel for nn_EfficientSpatioTemporalBlock.

Sharding: 8 cores = (batch 4) x (H halves 2). Per-core shard: one sample,
32 own H rows (+1 halo row each side). All intermediates live in SBUF (bf16)
in one in-place buffer per 128-channel block. Cross-half InstanceNorm / SE
stats are combined with tiny pairwise AllReduces.

Pipeline per core:
  stage1  1x1 conv (PE, fp32r)  -> A (bf16, raw g1)
  stats1  (pair allreduce) -> m1, r1 = rsqrt(var+eps)
  stage2  relu(g1 - m1) -> padded N tiles; 3x3 depthwise as 9 diagonal
          matmuls on PE (PSUM accum); copy *r1 -> A (g2)
  stats2 -> stage3 temporal 3-tap on DVE (weights pre-scaled by r2) -> A (g3)
  stats3 -> h3 = relu(g3 - m3) in-place, SE pool rides accum_out
  SE MLP (PE + ACT sigmoid), fold y*r3 into proj weights
  proj (PE, bf16) -> p staged to DRAM (bf16), stats4
  final: (p*r4 - m4*r4) + x, 2x2 maxpool, DMA out
"""

import sys

sys.path.insert(0, "/opt/trn_rl_repo")

import numpy as np
import ml_dtypes

import concourse.bass as bass
import concourse.bacc as bacc
import concourse.mybir as mybir
from concourse.tile import TileContext
from concourse.bass_utils import run_bass_kernel_spmd

F32 = mybir.dt.float32
F32R = mybir.dt.float32r
BF16 = mybir.dt.bfloat16
AX = mybir.AxisListType
OP = mybir.AluOpType
AF = mybir.ActivationFunctionType

CIN, HID, CO = 64, 256, 64
T, H, W = 16, 64, 64
YS, YH = 32, 34  # own rows, rows with halo
NPIX = float(T * H * W)
EPS = 1e-5
DEAD_M = 1e30

# y tiles for stage1 (halo-aligned): (y0, rows)
Y_TILES1 = [(0, 1), (1, 8), (9, 8), (17, 8), (25, 8), (33, 1)]
Y_OWN = {1, 9, 17, 25}


def _build_nc():
    nc = bass.Bass()

    xs = nc.declare_dram_parameter("xs", [CIN, T, YH, W], F32, isOutput=False)
    xs16 = nc.declare_dram_parameter("xs16", [CIN, T, YH, W], BF16, isOutput=False)
    w1t = nc.declare_dram_parameter("w1t", [CIN, HID], BF16, isOutput=False)
    diag = nc.declare_dram_parameter("diag", [128, 18 * 128], BF16, isOutput=False)
    wt = nc.declare_dram_parameter("wt", [128, 6], F32, isOutput=False)
    wse1t = nc.declare_dram_parameter("wse1t", [128, 128], F32, isOutput=False)
    wse2t = nc.declare_dram_parameter("wse2t", [64, 256], F32, isOutput=False)
    wprojt = nc.declare_dram_parameter("wprojt", [128, 128], F32, isOutput=False)
    hs = nc.declare_dram_parameter("hs", [128, 2], F32, isOutput=False)
    out = nc.declare_dram_parameter("out", [CO, T, 16, 32], F32, isOutput=True)
    dbg = nc.declare_dram_parameter("dbg", [128, 48], F32, isOutput=True)

    pdram = nc.dram_tensor("pdram", [CO, T * YS * W], BF16)
    cc_i = [nc.dram_tensor(f"cc{i}i", [128, 4], F32) for i in range(5)]
    cc_o = [nc.dram_tensor(f"cc{i}o", [128, 4], F32) for i in range(5)]
    GROUPS = [[0, 1], [2, 3], [4, 5], [6, 7]]

    from contextlib import ExitStack
    with ExitStack() as stk:
        sb = lambda *a: stk.enter_context(nc.sbuf_tensor(*a))
        A0 = sb("A0", [128, T, YH, W], BF16)
        A1 = sb("A1", [128, T, YH, W], BF16)
        N0 = sb("N0", [128, YH, 68], BF16)
        N1 = sb("N1", [128, YH, 68], BF16)
        M0 = sb("M0", [128, YS, W], BF16)
        M1 = sb("M1", [128, YS, W], BF16)
        M2 = sb("M2", [128, YS, W], BF16)
        M3 = sb("M3", [128, YS, W], BF16)
        MZ = sb("MZ", [128, YS, W], BF16)
        SC = sb("SC", [128, 768], F32)
        SS = sb("SS", [128, 48], F32)
        w1sb = sb("w1sb", [CIN, HID], BF16)
        diagsb = sb("diagsb", [128, 18 * 128], BF16)
        wtsb = sb("wtsb", [128, 6], F32)
        wse1sb = sb("wse1sb", [128, 128], F32)
        wse2sb = sb("wse2sb", [64, 256], F32)
        wprojsb = sb("wprojsb", [128, 128], F32)
        wpb = sb("wpb", [128, 128], BF16)
        hssb = sb("hssb", [128, 2], F32)
        zsb = sb("zsb", [64, 1], F32)
        ccs = sb("ccs", [128, 4], F32)
        ccr = sb("ccr", [128, 4], F32)
        tc = stk.enter_context(TileContext(nc))
        xin_pool = stk.enter_context(tc.tile_pool(name="xin", bufs=3))
        ps_pool = stk.enter_context(tc.tile_pool(name="ps", bufs=4, space="PSUM"))
        psp_pool = stk.enter_context(tc.tile_pool(name="psp", bufs=2, space="PSUM"))
        pst_pool = stk.enter_context(tc.tile_pool(name="pst", bufs=2))
        fin_pool = stk.enter_context(tc.tile_pool(name="fin", bufs=2))
        acc_pool = stk.enter_context(tc.tile_pool(name="accp", bufs=2))
        A = [A0, A1]
        NR = [N0, N1]
        MR = [M0, M1, M2, M3]

        # SS scalar column map (per block b: +16*b)
        M1C, R1C, M2C, R2C, M3C, R3C = 0, 1, 2, 3, 4, 5
        WT0, WT1, WT2 = 6, 7, 8
        YA3, POOLC = 9, 10
        TP0, TP1 = 11, 12
        # shared cols
        M4C, R4C, S1F, TPS = 32, 33, 34, 35
        EPSC, ZEROC = 36, 37

        def ss(col, p0=0, p1=128):
            return SS[p0:p1, col:col + 1]

        # ---- load weights ----
        nc.sync.dma_start(out=w1sb[:, :], in_=w1t[:, :])
        nc.sync.dma_start(out=diagsb[:, :], in_=diag[:, :])
        nc.sync.dma_start(out=wtsb[:, :], in_=wt[:, :])
        nc.sync.dma_start(out=wse1sb[:, :], in_=wse1t[:, :])
        nc.sync.dma_start(out=wse2sb[:, :], in_=wse2t[:, :])
        nc.sync.dma_start(out=wprojsb[:, :], in_=wprojt[:, :])
        nc.sync.dma_start(out=hssb[:, :], in_=hs[:, :])
        nc.vector.memset(MZ[:, :, :], 0.0)
        nc.vector.memset(SS[:, EPSC:EPSC + 1], EPS)
        nc.vector.memset(SS[:, ZEROC:ZEROC + 1], 0.0)
        for Nt in NR:
            nc.vector.memset(Nt[:, :, 0:2], 0.0)
            nc.vector.memset(Nt[:, :, 66:68], 0.0)

        sc_used = {}

        def sc_col(group, base):
            c = base + sc_used.get(group, 0)
            sc_used[group] = sc_used.get(group, 0) + 1
            return c

        def reduce_cols(dst, group, base, p0=0, p1=128):
            n = sc_used[group]
            nc.vector.tensor_reduce(dst, SC[p0:p1, base:base + n], AX.X, OP.add)

        def stats_from(ccr_ap_s, ccr_ap_q, mcol, rcol, blk, p0=0, p1=128):
            # m = S/NPIX ; r = exp(-0.5*ln(S2/NPIX - m^2 + eps))
            b = 16 * blk
            nc.vector.tensor_scalar(ss(mcol + b, p0, p1), ccr_ap_s, 1.0 / NPIX, None, OP.mult)
            nc.vector.tensor_scalar(ss(TP0 + b, p0, p1), ccr_ap_q, 1.0 / NPIX, None, OP.mult)
            nc.vector.tensor_tensor(ss(TP1 + b, p0, p1), ss(mcol + b, p0, p1), ss(mcol + b, p0, p1), OP.mult)
            nc.vector.tensor_tensor(ss(TP0 + b, p0, p1), ss(TP0 + b, p0, p1), ss(TP1 + b, p0, p1), OP.subtract)
            nc.scalar.activation(ss(TP0 + b, p0, p1), ss(TP0 + b, p0, p1), AF.Ln,
                                 bias=ss(EPSC, p0, p1), scale=1.0)
            nc.scalar.activation(ss(rcol + b, p0, p1), ss(TP0 + b, p0, p1), AF.Exp,
                                 bias=ss(ZEROC, p0, p1), scale=-0.5)

        # SC column group bases
        B_S1S = (0, 64)          # stage1 sums, per blk
        B_S1Q = (128, 144)       # stage1 sq
        B_S2S = (160, 224)
        B_S2Q = (288, 304)
        B_S3S = (320, 336)
        B_S3Q = (352, 368)
        B_PL = (384, 400)
        B_S4S = 416
        B_S4Q = 480

        # ================= stage 1: 1x1 conv =================
        for f in range(T):
            for (y0, rows) in Y_TILES1:
                n = rows * W
                xt = xin_pool.tile([CIN, n], BF16)
                nc.sync.dma_start(
                    out=xt[:, :],
                    in_=xs16[:, f, y0:y0 + rows, :].rearrange("c a b -> c (a b)"))
                for blk in range(2):
                    pt = ps_pool.tile([128, n], F32)
                    nc.tensor.matmul(
                        pt[:, :],
                        w1sb[:, blk * 128:(blk + 1) * 128],
                        xt[:, :],
                        start=True, stop=True,
                    )
                    dst = A[blk][:, f, y0:y0 + rows, :].rearrange("p a b -> p (a b)")
                    if y0 in Y_OWN:
                        c = sc_col(("s1s", blk), B_S1S[blk])
                        nc.scalar.activation(dst, pt[:, :], AF.Copy,
                                             accum_out=SC[:, c:c + 1])
                    else:
                        nc.scalar.activation(dst, pt[:, :], AF.Copy)

        # Sum of squares over own rows, from stored bf16
        for blk in range(2):
            for f in range(T):
                c = sc_col(("s1q", blk), B_S1Q[blk])
                src = A[blk][:, f, 1:33, :].rearrange("p a b -> p (a b)")
                scr = acc_pool.tile([128, YS * W], BF16, tag="acc3")
                nc.vector.scalar_tensor_tensor(
                    scr[:, :], src, 1.0, src,
                    OP.mult, OP.mult, accum_out=SC[:, c:c + 1])

        # stats1 allreduce (both blocks in one op)
        for blk in range(2):
            reduce_cols(ccs[:, 2 * blk:2 * blk + 1], ("s1s", blk), B_S1S[blk])
            reduce_cols(ccs[:, 2 * blk + 1:2 * blk + 2], ("s1q", blk), B_S1Q[blk])
        nc.sync.dma_start(out=cc_i[0][:, :], in_=ccs[:, :])
        nc.gpsimd.collective_compute(
            "AllReduce", OP.add, replica_groups=GROUPS,
            ins=[cc_i[0][:, :]], outs=[cc_o[0][:, :]])
        nc.sync.dma_start(out=ccr[:, :], in_=cc_o[0][:, :])
        for blk in range(2):
            stats_from(ccr[:, 2 * blk:2 * blk + 1], ccr[:, 2 * blk + 1:2 * blk + 2],
                       M1C, R1C, blk)
        # dead channels 32:64 (block 0): force normed value to 0
        nc.vector.memset(SS[32:64, M1C:M1C + 1], DEAD_M)

        # ================= stage 2: spatial 3x3 depthwise =================
        for blk in range(2):
            Ab = A[blk]
            m1 = ss(M1C + 16 * blk)
            for f in range(T):
                Nt = NR[f % 2]
                dst = Nt[:, :, 2:66]
                if blk == 0:
                    # identity channels 64:128 read frame f
                    nc.vector.tensor_scalar(
                        Nt[64:128, :, 2:66], Ab[64:128, f, :, :],
                        ss(M1C, 64, 128), 0.0, OP.subtract, OP.max)
                    # TIM shift channels 0:32 read frame f+1 (f=15 -> zero);
                    # dead channels 32:64 ride along (m1=1e30 forces 0)
                    if f < T - 1:
                        nc.vector.tensor_scalar(
                            Nt[0:64, :, 2:66], Ab[0:64, f + 1, :, :],
                            ss(M1C, 0, 64), 0.0, OP.subtract, OP.max)
                    else:
                        nc.vector.tensor_scalar(
                            Nt[0:64, :, 2:66], Ab[0:64, f, :, :],
                            0.0, 0.0, OP.mult, OP.mult)
                else:
                    nc.vector.tensor_scalar(
                        dst, Ab[:, f, :, :], m1, 0.0, OP.subtract, OP.max)
                # halo row masking (per-core top/bottom scale)
                nc.vector.tensor_scalar(
                    Nt[:, 0, 2:66], Nt[:, 0, 2:66], hssb[:, 0:1], None, OP.mult)
                nc.vector.tensor_scalar(
                    Nt[:, 33, 2:66], Nt[:, 33, 2:66], hssb[:, 1:2], None, OP.mult)

                for y0 in (0, 8, 16, 24):
                    pt = ps_pool.tile([128, 512], F32)
                    k = 0
                    for dy in range(3):
                        for dx in range(3):
                            nc.tensor.matmul(
                                pt[:, :],
                                diagsb[:, (blk * 9 + k) * 128:(blk * 9 + k + 1) * 128],
                                Nt[:, y0 + dy:y0 + dy + 8, 1 + dx:65 + dx],
                                start=(k == 0), stop=(k == 8))
                            k += 1
                    c = sc_col(("s2s", blk), B_S2S[blk])
                    nc.scalar.activation(
                        Ab[:, f, y0:y0 + 8, :].rearrange("p a b -> p (a b)"),
                        pt[:, :], AF.Copy,
                        scale=ss(R1C + 16 * blk), accum_out=SC[:, c:c + 1])

        # stats2
        for blk in range(2):
            for f in range(T):
                c = sc_col(("s2q", blk), B_S2Q[blk])
                src = A[blk][:, f, 0:32, :].rearrange("p a b -> p (a b)")
                scr = acc_pool.tile([128, YS * W], BF16, tag="acc3")
                nc.vector.scalar_tensor_tensor(
                    scr[:, :], src, 1.0, src,
                    OP.mult, OP.mult, accum_out=SC[:, c:c + 1])
        for blk in range(2):
            reduce_cols(ccs[:, 2 * blk:2 * blk + 1], ("s2s", blk), B_S2S[blk])
            reduce_cols(ccs[:, 2 * blk + 1:2 * blk + 2], ("s2q", blk), B_S2Q[blk])
        nc.sync.dma_start(out=cc_i[1][:, :], in_=ccs[:, :])
        nc.gpsimd.collective_compute(
            "AllReduce", OP.add, replica_groups=GROUPS,
            ins=[cc_i[1][:, :]], outs=[cc_o[1][:, :]])
        nc.sync.dma_start(out=ccr[:, :], in_=cc_o[1][:, :])
        for blk in range(2):
            stats_from(ccr[:, 2 * blk:2 * blk + 1], ccr[:, 2 * blk + 1:2 * blk + 2],
                       M2C, R2C, blk)

        # ================= stage 3: temporal 3-tap =================
        def g3_frame(blk, g):
            b = 16 * blk
            Ab = A[blk]
            mprev = MZ if g == 0 else MR[(g - 1) % 4]
            mnext = MZ if g == T - 1 else MR[(g + 1) % 4]
            acc = acc_pool.tile([128, YS * W], BF16, tag="acc3")
            nc.vector.tensor_scalar(
                acc[:, :], mprev[:, :, :].rearrange("p a b -> p (a b)"),
                ss(WT0 + b), None, OP.mult)
            nc.vector.scalar_tensor_tensor(
                acc[:, :], MR[g % 4][:, :, :].rearrange("p a b -> p (a b)"),
                ss(WT1 + b), acc[:, :], OP.mult, OP.add)
            c = sc_col(("s3s", blk), B_S3S[blk])
            nc.vector.scalar_tensor_tensor(
                Ab[:, g, 0:32, :].rearrange("p a b -> p (a b)"),
                mnext[:, :, :].rearrange("p a b -> p (a b)"),
                ss(WT2 + b), acc[:, :], OP.mult, OP.add,
                accum_out=SC[:, c:c + 1])

        for blk in range(2):
            b = 16 * blk
            # fold r2 into temporal tap weights
            for k in range(3):
                nc.vector.tensor_tensor(
                    ss(WT0 + k + b), wtsb[:, blk * 3 + k:blk * 3 + k + 1],
                    ss(R2C + b), OP.mult)
            Ab = A[blk]
            for f in range(T):
                nc.vector.tensor_scalar(
                    MR[f % 4][:, :, :], Ab[:, f, 0:32, :],
                    ss(M2C + b), 0.0, OP.subtract, OP.max)
                if f >= 1:
                    g3_frame(blk, f - 1)
            g3_frame(blk, T - 1)

        # stats3
        for blk in range(2):
            for f in range(T):
                c = sc_col(("s3q", blk), B_S3Q[blk])
                src = A[blk][:, f, 0:32, :].rearrange("p a b -> p (a b)")
                scr = acc_pool.tile([128, YS * W], BF16, tag="acc3")
                nc.vector.scalar_tensor_tensor(
                    scr[:, :], src, 1.0, src,
                    OP.mult, OP.mult, accum_out=SC[:, c:c + 1])
        for blk in range(2):
            reduce_cols(ccs[:, 2 * blk:2 * blk + 1], ("s3s", blk), B_S3S[blk])
            reduce_cols(ccs[:, 2 * blk + 1:2 * blk + 2], ("s3q", blk), B_S3Q[blk])
        nc.sync.dma_start(out=cc_i[2][:, :], in_=ccs[:, :])
        nc.gpsimd.collective_compute(
            "AllReduce", OP.add, replica_groups=GROUPS,
            ins=[cc_i[2][:, :]], outs=[cc_o[2][:, :]])
        nc.sync.dma_start(out=ccr[:, :], in_=cc_o[2][:, :])
        for blk in range(2):
            stats_from(ccr[:, 2 * blk:2 * blk + 1], ccr[:, 2 * blk + 1:2 * blk + 2],
                       M3C, R3C, blk)

        # ============ SE: h3 = relu(g3 - m3) in place, pool ============
        for blk in range(2):
            b = 16 * blk
            Ab = A[blk]
            for f in range(T):
                c = sc_col(("pl", blk), B_PL[blk])
                ap = Ab[:, f, 0:32, :].rearrange("p a b -> p (a b)")
                nc.vector.scalar_tensor_tensor(
                    ap, ap, ss(M3C + b), MZ[:, :, :].rearrange("p a b -> p (a b)"),
                    OP.subtract, OP.max, accum_out=SC[:, c:c + 1])
        for blk in range(2):
            reduce_cols(ccs[:, blk:blk + 1], ("pl", blk), B_PL[blk])
        nc.vector.memset(ccs[:, 2:4], 0.0)
        nc.sync.dma_start(out=cc_i[3][:, :], in_=ccs[:, :])
        nc.gpsimd.collective_compute(
            "AllReduce", OP.add, replica_groups=GROUPS,
            ins=[cc_i[3][:, :]], outs=[cc_o[3][:, :]])
        nc.sync.dma_start(out=ccr[:, :], in_=cc_o[3][:, :])
        # pooled_blk = sum * r3 / NPIX
        for blk in range(2):
            b = 16 * blk
            nc.vector.tensor_scalar(
                ss(TPS), ss(R3C + b), 1.0 / NPIX, None, OP.mult)
            nc.vector.tensor_tensor(
                ss(POOLC + b), ccr[:, blk:blk + 1], ss(TPS), OP.mult)
        # SE MLP
        psz = psp_pool.tile([64, 1], F32, tag="se")
        for blk in range(2):
            nc.tensor.matmul(
                psz[:, :], wse1sb[:, blk * 64:(blk + 1) * 64],
                ss(POOLC + 16 * blk), start=(blk == 0), stop=(blk == 1))
        nc.scalar.activation(zsb[:, :], psz[:, :], AF.Relu, bias=ss(ZEROC, 0, 64))
        for blk in range(2):
            b = 16 * blk
            psy = psp_pool.tile([128, 1], F32, tag="se")
            nc.tensor.matmul(
                psy[:, :], wse2sb[:, blk * 128:(blk + 1) * 128], zsb[:, :],
                start=True, stop=True)
            nc.scalar.activation(ss(TP0 + b), psy[:, :], AF.Sigmoid, bias=ss(ZEROC))
            # ya3 = y * r3 ; wp = w_projT * ya3  (bf16)
            nc.vector.tensor_tensor(ss(YA3 + b), ss(TP0 + b), ss(R3C + b), OP.mult)
            nc.vector.tensor_scalar(
                wpb[:, blk * 64:(blk + 1) * 64], wprojsb[:, blk * 64:(blk + 1) * 64],
                ss(YA3 + b), None, OP.mult)

        # ================= proj =================
        for f in range(T):
            for y0 in (0, 8, 16, 24):
                pt = psp_pool.tile([64, 512], F32)
                for blk in range(2):
                    nc.tensor.matmul(
                        pt[:, :], wpb[:, blk * 64:(blk + 1) * 64],
                        A[blk][:, f, y0:y0 + 8, :].rearrange("p a b -> p (a b)"),
                        start=(blk == 0), stop=(blk == 1))
                stg = pst_pool.tile([64, 512], BF16)
                c = sc_col("s4s", B_S4S)
                nc.scalar.activation(stg[:, :], pt[:, :], AF.Copy,
                                     accum_out=SC[0:64, c:c + 1])
                c = sc_col("s4q", B_S4Q)
                scr = acc_pool.tile([128, YS * W], BF16, tag="acc3")
                nc.vector.scalar_tensor_tensor(
                    scr[0:64, 0:512], stg[:, :], 1.0, stg[:, :], OP.mult, OP.mult,
                    accum_out=SC[0:64, c:c + 1])
                base = (f * YS + y0) * W
                nc.sync.dma_start(out=pdram[:, base:base + 512], in_=stg[:, :])

        # stats4
        reduce_cols(ccs[0:64, 0:1], "s4s", B_S4S, 0, 64)
        reduce_cols(ccs[0:64, 1:2], "s4q", B_S4Q, 0, 64)
        nc.vector.memset(ccs[64:128, 0:2], 0.0)
        nc.vector.memset(ccs[:, 2:4], 0.0)
        nc.sync.dma_start(out=cc_i[4][:, :], in_=ccs[:, :])
        nc.gpsimd.collective_compute(
            "AllReduce", OP.add, replica_groups=GROUPS,
            ins=[cc_i[4][:, :]], outs=[cc_o[4][:, :]])
        nc.sync.dma_start(out=ccr[:, :], in_=cc_o[4][:, :])
        stats_from(ccr[0:64, 0:1], ccr[0:64, 1:2], M4C - 32, R4C - 32, 2, 0, 64)
        # s1f = -m4*r4
        nc.vector.tensor_tensor(ss(TPS, 0, 64), ss(M4C, 0, 64), ss(R4C, 0, 64), OP.mult)
        nc.vector.tensor_scalar(ss(S1F, 0, 64), ss(TPS, 0, 64), -1.0, None, OP.mult)

        # ================= final =================
        for f in range(T):
            for y0 in (0, 8, 16, 24):
                base = (f * YS + y0) * W
                ptile = pst_pool.tile([64, 512], BF16, tag="pin")
                nc.sync.dma_start(out=ptile[:, :], in_=pdram[:, base:base + 512])
                xt = xin_pool.tile([64, 512], F32, tag="xres")
                nc.sync.dma_start(
                    out=xt[:, :],
                    in_=xs[:, f, 1 + y0:1 + y0 + 8, :].rearrange("c a b -> c (a b)"))
                af = fin_pool.tile([64, 8, 64], F32, tag="af")
                nc.vector.tensor_scalar(
                    af[:, :, :].rearrange("p a b -> p (a b)"), ptile[:, :],
                    ss(R4C, 0, 64), ss(S1F, 0, 64), OP.mult, OP.add)
                nc.vector.tensor_tensor(
                    af[:, :, :].rearrange("p a b -> p (a b)"), af[:, :, :].rearrange("p a b -> p (a b)"),
                    xt[:, :], OP.add)
                a2 = af[:, :, :].rearrange("p y (x t) -> p y x t", t=2)
                mp1 = fin_pool.tile([64, 8, 32], F32, tag="mp1")
                nc.vector.tensor_tensor(mp1[:, :, :], a2[:, :, :, 0], a2[:, :, :, 1], OP.max)
                b2 = mp1[:, :, :].rearrange("p (y t) x -> p y t x", t=2)
                mp2 = fin_pool.tile([64, 4, 32], F32, tag="mp2")
                nc.vector.tensor_tensor(mp2[:, :, :], b2[:, 0:4, 0, :], b2[:, 0:4, 1, :], OP.max)
                nc.sync.dma_start(out=out[:, f, y0 // 2:y0 // 2 + 4, :], in_=mp2[:, :, :])
        nc.sync.dma_start(out=dbg[:, :], in_=SS[:, :])

    import bass_rust as _br
    _br.move_matmul_waits_to_ldweights(nc.m)
    _br.generate_event_semaphores(nc)
    return nc


_CACHE = {}


def kernel(x, w1, w_dw_s, w_dw_t, w_se1, w_se2, w_proj):
    x = np.ascontiguousarray(x, np.float32)
    B = x.shape[0]
    if "nc" not in _CACHE:
        _CACHE["nc"] = _build_nc()
    nc = _CACHE["nc"]

    # host-side input prep
    xpad = np.zeros((B, CIN, T, H + 2, W), np.float32)
    xpad[:, :, :, 1:65, :] = x
    w1t = np.ascontiguousarray(w1.T.astype(ml_dtypes.bfloat16))
    diag = np.zeros((128, 18, 128), ml_dtypes.bfloat16)
    idx = np.arange(128)
    for blk in range(2):
        k = 0
        for dy in range(3):
            for dx in range(3):
                diag[idx, blk * 9 + k, idx] = w_dw_s[blk * 128:(blk + 1) * 128, 0, 0, dy, dx].astype(
                    ml_dtypes.bfloat16)
                k += 1
    diag = np.ascontiguousarray(diag.reshape(128, 18 * 128))
    wt = np.zeros((128, 6), np.float32)
    for blk in range(2):
        for k in range(3):
            wt[:, blk * 3 + k] = w_dw_t[blk * 128:(blk + 1) * 128, 0, k, 0, 0]
    wse1t = np.concatenate([w_se1[:, :128].T, w_se1[:, 128:].T], axis=1).astype(np.float32)
    wse2t = np.ascontiguousarray(w_se2.T, np.float32)
    wprojt = np.concatenate([w_proj[:, :128].T, w_proj[:, 128:].T], axis=1).astype(np.float32)
    wse1t = np.ascontiguousarray(wse1t)
    wprojt = np.ascontiguousarray(wprojt)

    in_maps = []
    for core in range(8):
        b, j = core // 2, core % 2
        hsv = np.ones((128, 2), np.float32)
        if j == 0:
            hsv[:, 0] = 0.0
        else:
            hsv[:, 1] = 0.0
        in_maps.append({
            "xs": np.ascontiguousarray(xpad[b, :, :, 32 * j:32 * j + 34, :]),
            "xs16": np.ascontiguousarray(
                xpad[b, :, :, 32 * j:32 * j + 34, :].astype(ml_dtypes.bfloat16)),
            "w1t": w1t,
            "diag": diag,
            "wt": wt,
            "wse1t": wse1t,
            "wse2t": wse2t,
            "wprojt": wprojt,
            "hs": hsv,
        })

    res = run_bass_kernel_spmd(nc, in_maps, core_ids=list(range(8)))
    _CACHE["exec_time_ns"] = getattr(res, "exec_time_ns", None)
    _CACHE["results"] = res.results
    out = np.zeros((B, CO, T, 32, 32), np.float32)
    for core in range(8):
        b, j = core // 2, core % 2
        out[b, :, :, 16 * j:16 * j + 16, :] = res.results[core]["out"]
    return out

